# revision 3
# baseline (speedup 1.0000x reference)
"""Bass/Trainium2 kernel for nn_BaselineAttention — loop-reps variant.

Same dataflow as the baseline kernel (DPx2 over batch, TPx4 over heads,
transposed activations, flash-style softmax via ones-column in V, LN1 stats
AllReduce, fc ReduceScatter, LN2 on scattered rows), but the timing reps are
a single `tc.For_i` hardware loop around one static copy of the body instead
of n_reps unrolled copies.  All APs in the body are static, so re-executions
run from the instruction cache instead of paying the per-instruction
first-execution cost this runtime charges.
"""

import contextlib

import numpy as np

import concourse.bacc as bacc
import concourse.mybir as mybir
import concourse.tile as tile
from concourse.bass_utils import run_bass_kernel_spmd

F32 = mybir.dt.float32
F32R = mybir.dt.float32r
AF = mybir.ActivationFunctionType
OP = mybir.AluOpType
AX = mybir.AxisListType

B, S, D, H = 2, 2048, 1024, 16
EPS = 1e-3
SCALE = 0.125            # 1/sqrt(D/H)
GROUPS = [[0, 1, 2, 3], [4, 5, 6, 7]]

_BUILD_CACHE = {}


def _build(n_reps=1):
    key = n_reps
    if key in _BUILD_CACHE:
        return _BUILD_CACHE[key]
    r1 = rar = r2 = rrs = r3 = n_reps

    nc = bacc.Bacc("TRN2", target_bir_lowering=False, debug=False, num_devices=8)

    xt_d = nc.dram_tensor("xt", [128, 8, S], F32R, kind="ExternalInput").ap()
    wq_d = nc.dram_tensor("wq", [128, 8, 128], F32R, kind="ExternalInput").ap()
    wk_d = nc.dram_tensor("wk", [128, 8, 128], F32R, kind="ExternalInput").ap()
    wv_d = nc.dram_tensor("wv", [128, 8, 256], F32R, kind="ExternalInput").ap()
    wfc_d = nc.dram_tensor("wfc", [128, 2, D], F32R, kind="ExternalInput").ap()
    onesrow_d = nc.dram_tensor("onesrow", [1, 128], F32R, kind="ExternalInput").ap()
    onesmat_d = nc.dram_tensor("onesmat", [128, 128], F32R, kind="ExternalInput").ap()
    onescol_d = nc.dram_tensor("onescol", [128, 1], F32R, kind="ExternalInput").ap()
    onesv_d = nc.dram_tensor("onesv", [128, 64], F32R, kind="ExternalInput").ap()
    ident_d = nc.dram_tensor("ident", [128, 128], F32R, kind="ExternalInput").ap()
    bq_d = nc.dram_tensor("bq", [128, 1], F32, kind="ExternalInput").ap()
    bk_d = nc.dram_tensor("bk", [128, 1], F32, kind="ExternalInput").ap()
    bv_d = nc.dram_tensor("bv", [128, 2], F32, kind="ExternalInput").ap()
    g1_d = nc.dram_tensor("g1", [128, 2], F32, kind="ExternalInput").ap()
    b1_d = nc.dram_tensor("b1", [128, 2], F32, kind="ExternalInput").ap()
    g2row_d = nc.dram_tensor("g2row", [1, D], F32R, kind="ExternalInput").ap()
    b2row_d = nc.dram_tensor("b2row", [1, D], F32R, kind="ExternalInput").ap()
    bfcrow_d = nc.dram_tensor("bfcrow", [1, D], F32R, kind="ExternalInput").ap()
    out_d = nc.dram_tensor("out", [4, 128, D], F32, kind="ExternalOutput").ap()

    with (
        tile.TileContext(nc) as tc,
        tc.tile_pool(name="sb", bufs=1) as sb,
        tc.tile_pool(name="ps", bufs=1, space="PSUM") as ps,
        tc.tile_pool(name="dr", bufs=1, space="DRAM") as dr,
    ):
        onesrow = sb.tile([1, 128], F32R)
        onesmat = sb.tile([128, 128], F32R)
        onescol = sb.tile([128, 1], F32R)
        ident = sb.tile([128, 128], F32R)
        wq = sb.tile([128, 8, 128], F32R)
        wk = sb.tile([128, 8, 128], F32R)
        wv = sb.tile([128, 8, 256], F32R)
        wfc = sb.tile([128, 2, D], F32R)
        bq = sb.tile([128, 1], F32)
        bk = sb.tile([128, 1], F32)
        bv = sb.tile([128, 2], F32)
        g1 = sb.tile([128, 2], F32)
        b1 = sb.tile([128, 2], F32)
        for t, d in [(onesrow, onesrow_d), (onesmat, onesmat_d),
                     (onescol, onescol_d), (ident, ident_d),
                     (wq, wq_d), (wk, wk_d), (wv, wv_d), (wfc, wfc_d),
                     (bq, bq_d), (bk, bk_d), (bv, bv_d), (g1, g1_d), (b1, b1_d)]:
            nc.sync.dma_start(t[:], d[:])

        # broadcast gamma2 / beta2 / bfc rows to [128, D] once (the [1, D]
        # staging row cycles one shared buffer — startup only)
        g2bc = sb.tile([128, D], F32)
        b2bc = sb.tile([128, D], F32)
        fcbc = sb.tile([128, D], F32)
        for row_d, dst in [(g2row_d, g2bc), (b2row_d, b2bc), (bfcrow_d, fcbc)]:
            row = sb.tile([1, D], F32R, tag="prow", bufs=1, name=f"row_{dst.name}")
            nc.sync.dma_start(row[:], row_d[:])
            bc_ps = ps.tile([128, 2048], F32, tag="tagA", bufs=1, name=f"bc_{dst.name}")
            for nch in range(2):
                nc.tensor.matmul(bc_ps[:, 512 * nch:512 * nch + 512],
                                 onesrow[:], row[0:1, 512 * nch:512 * nch + 512],
                                 start=True, stop=True)
            nc.vector.tensor_copy(dst[:], bc_ps[:, 0:1024])

        # persistent state
        vnat = sb.tile([128, 16, 260], F32R)   # V natural + ones cols
        nc.sync.dma_start(
            vnat[:].rearrange("p t (h x) -> p (t h) x", h=4)[:, :, 64:65],
            onesv_d[:].unsqueeze(2),
        )
        qt_sb = sb.tile([128, S], F32R)        # [qk-feat, tok]
        kt_sb = sb.tile([128, S], F32R)
        ysb_t = sb.tile([128, 2, S], F32R)     # [vfeat-local, jj, tok]
        ysb = ysb_t[:]
        ut_sb = ysb_t[:].bitcast(F32)          # f32 view for DVE reads
        yn_t = sb.tile([128, 2, S], F32R)      # LN1-normalized copy of ysb
        yn = yn_t[:]
        ynf = yn_t[:].bitcast(F32)

        stats_in = dr.tile([2, S], F32R)       # [sum; sumsq]
        stats_out = dr.tile([2, S], F32R)
        rs_in = dr.tile([S, D], F32)
        rs_out = dr.tile([512, D], F32)

        def loop(name, r):
            return (tc.For_i(0, r, 1, name=name) if r > 1
                    else contextlib.nullcontext())

        with loop("reploop1", r1):
            # ---------------- P1: projections ----------------
            for half in range(4):
                xs = sb.tile([128, 8, 512], F32R, tag="xs", bufs=1,
                             name=f"xs{half}")
                nc.sync.dma_start(xs[:], xt_d[:, :, 512 * half:512 * half + 512])
                for pname, w_t, mcol, bias, dst in [
                    ("q", wq, None, bq[:], qt_sb[:]),
                    ("k", wk, None, bk[:], kt_sb[:]),
                    ("v0", wv, slice(0, 128), bv[:, 0:1], None),
                    ("v1", wv, slice(128, 256), bv[:, 1:2], None),
                ]:
                    p_t = ps.tile([128, 512], F32,
                                  tag="tagA" if pname in ("q", "v0") else "tagB",
                                  bufs=1, name=f"p{pname}{half}")
                    for kc in range(8):
                        nc.tensor.matmul(
                            p_t[:],
                            w_t[:, kc, :] if mcol is None else w_t[:, kc, mcol],
                            xs[:, kc, :],
                            start=(kc == 0), stop=(kc == 7))
                    if pname in ("q", "k"):
                        nc.vector.tensor_scalar(
                            dst[:, 512 * half:512 * half + 512], p_t[:],
                            bias, None, OP.add)
                    else:
                        jj = 0 if pname == "v0" else 1
                        vt_st = sb.tile([128, 512], F32R, tag="vtst", bufs=1,
                                        name=f"vt{jj}{half}")
                        nc.vector.tensor_scalar(vt_st[:], p_t[:], bias, None, OP.add)
                        t_ps = ps.tile([128, 512], F32,
                                       tag="tagA" if pname == "v0" else "tagB",
                                       bufs=1, name=f"t{jj}{half}")
                        for blk in range(4):
                            nc.tensor.transpose(
                                t_ps[:, 128 * blk:128 * blk + 128].bitcast(F32R),
                                vt_st[:, 128 * blk:128 * blk + 128], ident[:])
                        nc.vector.tensor_copy(
                            vnat[:, 4 * half:4 * half + 4, :]
                            .rearrange("p t (h x) -> p t h x", h=4)
                            [:, :, 2 * jj:2 * jj + 2, 0:64],
                            t_ps[:].rearrange("p (t h x) -> p t h x", t=4, h=2),
                        )

            # ---------------- P2: attention (+ per-head d broadcast) --------
            dstages = []
            for jj in range(2):
                dstages.append(sb.tile([128, 2048], F32, tag="ln1t", bufs=3,
                                       name=f"dst{jj}"))
            for hl in range(4):
                s_ps = ps.tile([128, 2048], F32, tag="tagA", bufs=1,
                               name=f"s{hl}")
                u_ps = ps.tile([65, 2048], F32, tag="tagB", bufs=1,
                               name=f"u{hl}")
                e_t = sb.tile([128, 2048], F32R, tag="e", bufs=1, name=f"e{hl}")
                # ping-pong the two 1024-wide halves of s_ps/e_t so the PE's
                # logits matmuls of one half-chunk overlap the scalar
                # engine's exp of the previous one (subtile deps keep the
                # WAR hazards per-half instead of per-tile)
                for kc in range(16):
                    for uh in range(2):
                        pp = 1024 * uh
                        for uu in range(2):
                            u = 2 * uh + uu
                            nc.tensor.matmul(
                                s_ps[:, pp + 512 * uu:pp + 512 * uu + 512],
                                kt_sb[32 * hl:32 * hl + 32,
                                      128 * kc:128 * kc + 128],
                                qt_sb[32 * hl:32 * hl + 32,
                                      512 * u:512 * u + 512],
                                tile_position=(32 * hl, 0),
                                start=True, stop=True)
                        nc.scalar.activation(e_t[:, pp:pp + 1024],
                                             s_ps[:, pp:pp + 1024],
                                             AF.Exp, scale=SCALE)
                        for uu in range(2):
                            u = 2 * uh + uu
                            nc.tensor.matmul(
                                u_ps[:, 512 * u:512 * u + 512],
                                vnat[:, kc, 65 * hl:65 * hl + 65],
                                e_t[:, pp + 512 * uu:pp + 512 * uu + 512],
                                start=(kc == 0), stop=(kc == 15))
                u_st = sb.tile([65, S], F32R, tag="ust", bufs=2, name=f"ust{hl}")
                nc.vector.tensor_copy(u_st[:], u_ps[:])
                nc.sync.dma_start(
                    ysb[64 * (hl % 2):64 * (hl % 2) + 64, hl // 2, :],
                    u_st[0:64, :])
                # broadcast this head's denominator row (at partition 64)
                db = ps.tile([128, 2048], F32, tag="tagA", bufs=1,
                             name=f"db{hl}")
                for u in range(4):
                    nc.tensor.matmul(
                        db[:, 512 * u:512 * u + 512], onesmat[64:65, :],
                        u_st[64:65, 512 * u:512 * u + 512], start=True, stop=True)
                half = hl % 2
                nc.vector.tensor_copy(
                    dstages[hl // 2][64 * half:64 * half + 64, :],
                    db[64 * half:64 * half + 64, :])

            # ---------------- P3: divide, LN1 stats + AR, normalize ----------
            for jj in range(2):
                rec_t = sb.tile([128, 2048], F32, tag="ln1t", bufs=3,
                                name=f"rec{jj}")
                nc.vector.reciprocal_approx_fast(rec_t[:], dstages[jj][:])
                nc.vector.tensor_tensor(ysb[:, jj, :], ut_sb[:, jj, :], rec_t[:],
                                        OP.mult)
            st_s = ps.tile([1, 2048], F32, tag="tagA", bufs=1, name="sts")
            st_q = ps.tile([1, 2048], F32, tag="tagB", bufs=1, name="stq")
            for jj in range(2):
                ysq = sb.tile([128, 2048], F32R, tag="ln1t", bufs=3,
                              name=f"ysq{jj}")
                nc.vector.tensor_tensor(ysq[:], ysb[:, jj, :], ysb[:, jj, :], OP.mult)
                for u in range(4):
                    usl = slice(512 * u, 512 * u + 512)
                    nc.tensor.matmul(st_s[0:1, usl], onescol[:], ysb[:, jj, usl],
                                     start=(jj == 0), stop=(jj == 1))
                    nc.tensor.matmul(st_q[0:1, usl], onescol[:], ysq[:, usl],
                                     start=(jj == 0), stop=(jj == 1))
            ss_st = sb.tile([1, 2048], F32R, tag="row", bufs=2, name="ssst")
            sq_st = sb.tile([1, 2048], F32R, tag="row", bufs=2, name="sqst")
            nc.vector.tensor_copy(ss_st[:], st_s[0:1, :])
            nc.vector.tensor_copy(sq_st[:], st_q[0:1, :])
            nc.sync.dma_start(stats_in[0:1, :], ss_st[:])
            nc.sync.dma_start(stats_in[1:2, :], sq_st[:])

        # collectives cannot live inside a For_i loop on this runtime
        # (NRT_EXEC_UNIT_UNRECOVERABLE), so the timing reps run them
        # unrolled between the loops; every rep's data is identical, so
        # results are unchanged and each rep still executes them once.
        for _ in range(rar):
            nc.gpsimd.collective_compute(
                "AllReduce", OP.add, replica_groups=GROUPS,
                ins=[stats_in[:]], outs=[stats_out[:]])

        with loop("reploop2", r2):
            str_s = sb.tile([1, 2048], F32R, tag="row", bufs=2, name="strs")
            str_q = sb.tile([1, 2048], F32R, tag="row", bufs=2, name="strq")
            nc.sync.dma_start(str_s[:], stats_out[0:1, :])
            nc.sync.dma_start(str_q[:], stats_out[1:2, :])

            bs_ps = ps.tile([128, 2048], F32, tag="tagA", bufs=1, name="bs")
            bq_ps = ps.tile([128, 2048], F32, tag="tagB", bufs=1, name="bq2")
            for u in range(4):
                usl = slice(512 * u, 512 * u + 512)
                nc.tensor.matmul(bs_ps[:, usl], onesrow[:], str_s[0:1, usl],
                                 start=True, stop=True)
                nc.tensor.matmul(bq_ps[:, usl], onesrow[:], str_q[0:1, usl],
                                 start=True, stop=True)
            t_mu = sb.tile([128, 2048], F32, tag="ln1t", bufs=3, name="tmu")
            t_v = sb.tile([128, 2048], F32, tag="ln1t", bufs=3, name="tv")
            t_w = sb.tile([128, 2048], F32, tag="ln1t", bufs=3, name="tw")
            nc.vector.tensor_scalar(t_mu[:], bs_ps[:], 1.0 / D, None, OP.mult)
            nc.vector.tensor_scalar(t_v[:], bq_ps[:], 1.0 / D, None, OP.mult)
            nc.vector.tensor_tensor(t_w[:], t_mu[:], t_mu[:], OP.mult)
            nc.vector.tensor_tensor(t_v[:], t_v[:], t_w[:], OP.subtract)
            nc.vector.tensor_scalar(t_v[:], t_v[:], EPS, None, OP.add)
            nc.vector.reciprocal_approx_fast(t_w[:], t_v[:])
            nc.scalar.activation(t_v[:], t_w[:], AF.Sqrt)                 # r
            nc.vector.tensor_tensor(t_w[:], t_mu[:], t_v[:], OP.mult)     # mu*r
            for jj in range(2):
                nc.vector.tensor_tensor(yn[:, jj, :], ut_sb[:, jj, :], t_v[:], OP.mult)
                nc.vector.tensor_tensor(yn[:, jj, :], ynf[:, jj, :], t_w[:],
                                        OP.subtract)
                nc.vector.tensor_scalar(yn[:, jj, :], ynf[:, jj, :],
                                        g1[:, jj:jj + 1], b1[:, jj:jj + 1],
                                        OP.mult, OP.add)

            # ---------------- fc + RS ----------------
            for pair in range(8):       # 2 token-chunks of 128 per psum tile
                fc_ps = ps.tile([128, 2048], F32,
                                tag="tagA" if pair % 2 == 0 else "tagB",
                                bufs=1, name=f"fc{pair}")
                for half in range(2):
                    tok = slice(256 * pair + 128 * half, 256 * pair + 128 * half + 128)
                    for jj in range(2):
                        for nch in range(2):
                            nc.tensor.matmul(
                                fc_ps[:, 1024 * half + 512 * nch:
                                      1024 * half + 512 * nch + 512],
                                yn[:, jj, tok],
                                wfc[:, jj, 512 * nch:512 * nch + 512],
                                start=(jj == 0), stop=(jj == 1))
                p_st = sb.tile([128, 2048], F32, tag="pst", bufs=1,
                               name=f"pst{pair}")
                nc.vector.tensor_copy(p_st[:], fc_ps[:])
                nc.sync.dma_start(
                    rs_in[256 * pair:256 * pair + 256, :]
                    .rearrange("(t p) n -> p t n", t=2),
                    p_st[:].rearrange("p (t n) -> p t n", t=2))

        for _ in range(rrs):
            nc.gpsimd.collective_compute(
                "ReduceScatter", OP.add, replica_groups=GROUPS,
                ins=[rs_in[:]], outs=[rs_out[:]])

        # ---------------- LN2 ----------------
        with loop("reploop3", r3):
            for ts in range(4):
                pP = sb.tile([128, D], F32, tag="pP", bufs=1, name=f"pP{ts}")
                nc.sync.dma_start(pP[:], rs_out[128 * ts:128 * ts + 128, :])
                nc.vector.tensor_tensor(pP[:], pP[:], fcbc[:], OP.add)
                s2 = sb.tile([128, 8], F32, tag="s2", bufs=2, name=f"s2{ts}")
                nc.vector.tensor_reduce(s2[:, 0:1], pP[:], AX.X, OP.add)
                sqd = sb.tile([128, D], F32, tag="ln1t", bufs=3, name=f"sqd{ts}")
                nc.scalar.activation(sqd[:], pP[:], AF.Square, accum_out=s2[:, 1:2])
                nc.vector.tensor_scalar(s2[:, 0:1], s2[:, 0:1], 1.0 / D, None, OP.mult)
                nc.vector.tensor_scalar(s2[:, 1:2], s2[:, 1:2], 1.0 / D, None, OP.mult)
                nc.vector.tensor_tensor(s2[:, 2:3], s2[:, 0:1], s2[:, 0:1], OP.mult)
                nc.vector.tensor_tensor(s2[:, 3:4], s2[:, 1:2], s2[:, 2:3], OP.subtract)
                nc.vector.tensor_scalar(s2[:, 3:4], s2[:, 3:4], EPS, None, OP.add)
                nc.vector.reciprocal_approx_fast(s2[:, 4:5], s2[:, 3:4])
                nc.scalar.activation(s2[:, 5:6], s2[:, 4:5], AF.Sqrt)
                nc.vector.tensor_scalar(pP[:], pP[:], s2[:, 0:1], s2[:, 5:6],
                                        OP.subtract, OP.mult)
                nc.vector.tensor_tensor(pP[:], pP[:], g2bc[:], OP.mult)
                nc.vector.tensor_tensor(pP[:], pP[:], b2bc[:], OP.add)
                nc.sync.dma_start(out_d[ts], pP[:])

    nc.compile()
    _BUILD_CACHE[key] = nc
    return nc


def make_in_maps(x, Wq, bq, Wk, bk, Wv, bv, gamma1, beta1, Wfc, bfc, gamma2, beta2):
    x = np.asarray(x, np.float32)
    in_maps = []
    onesrow = np.ones((1, 128), np.float32)
    onesmat = np.ones((128, 128), np.float32)
    onescol = np.ones((128, 1), np.float32)
    onesv = np.ones((128, 64), np.float32)
    ident = np.eye(128, dtype=np.float32)
    Wq, Wk, Wv, Wfc = (np.asarray(a, np.float32) for a in (Wq, Wk, Wv, Wfc))
    for c in range(8):
        g, r = c // 4, c % 4
        xt = np.ascontiguousarray(
            x[g].T.reshape(8, 128, S).transpose(1, 0, 2))          # [128, 8, S]
        wq_c = np.ascontiguousarray(
            Wq[:, 128 * r:128 * r + 128].reshape(8, 128, 128).transpose(1, 0, 2))
        wk_c = np.ascontiguousarray(
            Wk[:, 128 * r:128 * r + 128].reshape(8, 128, 128).transpose(1, 0, 2))
        wv_c = np.ascontiguousarray(
            Wv[:, 256 * r:256 * r + 256].reshape(8, 128, 256).transpose(1, 0, 2))
        wfc_c = np.ascontiguousarray(
            Wfc[256 * r:256 * r + 256, :].reshape(2, 128, D).transpose(1, 0, 2))
        in_maps.append({
            "xt": xt, "wq": wq_c, "wk": wk_c, "wv": wv_c, "wfc": wfc_c,
            "onesrow": onesrow, "onesmat": onesmat, "onescol": onescol,
            "onesv": onesv, "ident": ident,
            "bq": np.asarray(bq, np.float32)[128 * r:128 * r + 128, None],
            "bk": np.asarray(bk, np.float32)[128 * r:128 * r + 128, None],
            "bv": np.asarray(bv, np.float32)[256 * r:256 * r + 256]
                 .reshape(2, 128).T.copy(),
            "g1": np.asarray(gamma1, np.float32)[256 * r:256 * r + 256]
                 .reshape(2, 128).T.copy(),
            "b1": np.asarray(beta1, np.float32)[256 * r:256 * r + 256]
                 .reshape(2, 128).T.copy(),
            "g2row": np.asarray(gamma2, np.float32)[None, :].copy(),
            "b2row": np.asarray(beta2, np.float32)[None, :].copy(),
            "bfcrow": np.asarray(bfc, np.float32)[None, :].copy(),
        })
    return in_maps


def assemble(results):
    out = np.empty((B, S, D), np.float32)
    for c in range(8):
        g, r = c // 4, c % 4
        o = results[c]["out"]                   # [4, 128, D] = slab r of batch g
        for ts in range(4):
            out[g, 512 * r + 128 * ts:512 * r + 128 * ts + 128, :] = o[ts]
    return out


def kernel(**inputs):
    nc = _build()
    in_maps = make_in_maps(**{k: np.asarray(v) for k, v in inputs.items()})
    res = run_bass_kernel_spmd(nc, in_maps, list(range(8)))
    return assemble(res.results)


# revision 4
# speedup vs baseline: 1.0363x; 1.0363x over previous
"""Bass/Trainium2 kernel for nn_BaselineAttention — loop-reps variant.

Same dataflow as the baseline kernel (DPx2 over batch, TPx4 over heads,
transposed activations, flash-style softmax via ones-column in V, LN1 stats
AllReduce, fc ReduceScatter, LN2 on scattered rows), but the timing reps are
a single `tc.For_i` hardware loop around one static copy of the body instead
of n_reps unrolled copies.  All APs in the body are static, so re-executions
run from the instruction cache instead of paying the per-instruction
first-execution cost this runtime charges.
"""

import contextlib

import numpy as np

import concourse.bacc as bacc
import concourse.mybir as mybir
import concourse.tile as tile
from concourse.bass_utils import run_bass_kernel_spmd

F32 = mybir.dt.float32
F32R = mybir.dt.float32r
AF = mybir.ActivationFunctionType
OP = mybir.AluOpType
AX = mybir.AxisListType

B, S, D, H = 2, 2048, 1024, 16
EPS = 1e-3
SCALE = 0.125            # 1/sqrt(D/H)
GROUPS = [[0, 1, 2, 3], [4, 5, 6, 7]]

_BUILD_CACHE = {}


def _build(n_reps=1):
    key = n_reps
    if key in _BUILD_CACHE:
        return _BUILD_CACHE[key]
    r1 = rar = r2 = rrs = r3 = n_reps

    nc = bacc.Bacc("TRN2", target_bir_lowering=False, debug=False, num_devices=8)

    xt_d = nc.dram_tensor("xt", [128, 8, S], F32R, kind="ExternalInput").ap()
    wq_d = nc.dram_tensor("wq", [128, 8, 128], F32R, kind="ExternalInput").ap()
    wk_d = nc.dram_tensor("wk", [128, 8, 128], F32R, kind="ExternalInput").ap()
    wv_d = nc.dram_tensor("wv", [128, 8, 256], F32R, kind="ExternalInput").ap()
    wfc_d = nc.dram_tensor("wfc", [128, 2, D], F32R, kind="ExternalInput").ap()
    onesrow_d = nc.dram_tensor("onesrow", [1, 128], F32R, kind="ExternalInput").ap()
    onesmat_d = nc.dram_tensor("onesmat", [128, 128], F32R, kind="ExternalInput").ap()
    onescol_d = nc.dram_tensor("onescol", [128, 1], F32R, kind="ExternalInput").ap()
    onesv_d = nc.dram_tensor("onesv", [128, 64], F32R, kind="ExternalInput").ap()
    ident_d = nc.dram_tensor("ident", [128, 128], F32R, kind="ExternalInput").ap()
    bq_d = nc.dram_tensor("bq", [128, 1], F32, kind="ExternalInput").ap()
    bk_d = nc.dram_tensor("bk", [128, 1], F32, kind="ExternalInput").ap()
    bv_d = nc.dram_tensor("bv", [128, 2], F32, kind="ExternalInput").ap()
    g1_d = nc.dram_tensor("g1", [128, 2], F32, kind="ExternalInput").ap()
    b1_d = nc.dram_tensor("b1", [128, 2], F32, kind="ExternalInput").ap()
    g2row_d = nc.dram_tensor("g2row", [1, D], F32R, kind="ExternalInput").ap()
    b2row_d = nc.dram_tensor("b2row", [1, D], F32R, kind="ExternalInput").ap()
    bfcrow_d = nc.dram_tensor("bfcrow", [1, D], F32R, kind="ExternalInput").ap()
    out_d = nc.dram_tensor("out", [4, 128, D], F32, kind="ExternalOutput").ap()

    with (
        tile.TileContext(nc) as tc,
        tc.tile_pool(name="sb", bufs=1) as sb,
        tc.tile_pool(name="ps", bufs=1, space="PSUM") as ps,
        tc.tile_pool(name="dr", bufs=1, space="DRAM") as dr,
    ):
        onesrow = sb.tile([1, 128], F32R)
        onesmat = sb.tile([128, 128], F32R)
        onescol = sb.tile([128, 1], F32R)
        ident = sb.tile([128, 128], F32R)
        wq = sb.tile([128, 8, 128], F32R)
        wk = sb.tile([128, 8, 128], F32R)
        wv = sb.tile([128, 8, 256], F32R)
        wfc = sb.tile([128, 2, D], F32R)
        bq = sb.tile([128, 1], F32)
        bk = sb.tile([128, 1], F32)
        bv = sb.tile([128, 2], F32)
        g1 = sb.tile([128, 2], F32)
        b1 = sb.tile([128, 2], F32)
        for t, d in [(onesrow, onesrow_d), (onesmat, onesmat_d),
                     (onescol, onescol_d), (ident, ident_d),
                     (wq, wq_d), (wk, wk_d), (wv, wv_d), (wfc, wfc_d),
                     (bq, bq_d), (bk, bk_d), (bv, bv_d), (g1, g1_d), (b1, b1_d)]:
            nc.sync.dma_start(t[:], d[:])

        # broadcast gamma2 / beta2 / bfc rows to [128, D] once (the [1, D]
        # staging row cycles one shared buffer — startup only)
        g2bc = sb.tile([128, D], F32)
        b2bc = sb.tile([128, D], F32)
        fcbc = sb.tile([128, D], F32)
        for row_d, dst in [(g2row_d, g2bc), (b2row_d, b2bc), (bfcrow_d, fcbc)]:
            row = sb.tile([1, D], F32R, tag="prow", bufs=1, name=f"row_{dst.name}")
            nc.sync.dma_start(row[:], row_d[:])
            bc_ps = ps.tile([128, 2048], F32, tag="tagA", bufs=1, name=f"bc_{dst.name}")
            for nch in range(2):
                nc.tensor.matmul(bc_ps[:, 512 * nch:512 * nch + 512],
                                 onesrow[:], row[0:1, 512 * nch:512 * nch + 512],
                                 start=True, stop=True)
            nc.vector.tensor_copy(dst[:], bc_ps[:, 0:1024])

        # persistent state
        vnat = sb.tile([128, 16, 260], F32R)   # V natural + ones cols
        nc.sync.dma_start(
            vnat[:].rearrange("p t (h x) -> p (t h) x", h=4)[:, :, 64:65],
            onesv_d[:].unsqueeze(2),
        )
        qt_sb = sb.tile([128, S], F32R)        # [qk-feat, tok]
        kt_sb = sb.tile([128, S], F32R)
        ysb_t = sb.tile([128, 2, S], F32R)     # [vfeat-local, jj, tok]
        ysb = ysb_t[:]
        ut_sb = ysb_t[:].bitcast(F32)          # f32 view for DVE reads
        yn_t = sb.tile([128, 2, S], F32R)      # LN1-normalized copy of ysb
        yn = yn_t[:]
        ynf = yn_t[:].bitcast(F32)

        stats_in = dr.tile([2, S], F32R)       # [sum; sumsq]
        stats_out = dr.tile([2, S], F32R)
        rs_in = dr.tile([S, D], F32)
        rs_out = dr.tile([512, D], F32)

        def loop(name, r):
            return (tc.For_i(0, r, 1, name=name) if r > 1
                    else contextlib.nullcontext())

        with loop("reploop1", r1):
            # ---------------- P1: projections ----------------
            for half in range(4):
                xs = sb.tile([128, 8, 512], F32R, tag="xs", bufs=1,
                             name=f"xs{half}")
                nc.sync.dma_start(xs[:], xt_d[:, :, 512 * half:512 * half + 512])
                for pname, w_t, mcol, bias, dst in [
                    ("q", wq, None, bq[:], qt_sb[:]),
                    ("k", wk, None, bk[:], kt_sb[:]),
                    ("v0", wv, slice(0, 128), bv[:, 0:1], None),
                    ("v1", wv, slice(128, 256), bv[:, 1:2], None),
                ]:
                    p_t = ps.tile([128, 512], F32,
                                  tag="tagA" if pname in ("q", "v0") else "tagB",
                                  bufs=1, name=f"p{pname}{half}")
                    for kc in range(8):
                        nc.tensor.matmul(
                            p_t[:],
                            w_t[:, kc, :] if mcol is None else w_t[:, kc, mcol],
                            xs[:, kc, :],
                            start=(kc == 0), stop=(kc == 7))
                    if pname in ("q", "k"):
                        nc.vector.tensor_scalar(
                            dst[:, 512 * half:512 * half + 512], p_t[:],
                            bias, None, OP.add)
                    else:
                        jj = 0 if pname == "v0" else 1
                        vt_st = sb.tile([128, 512], F32R, tag="vtst", bufs=1,
                                        name=f"vt{jj}{half}")
                        nc.vector.tensor_scalar(vt_st[:], p_t[:], bias, None, OP.add)
                        t_ps = ps.tile([128, 512], F32,
                                       tag="tagA" if pname == "v0" else "tagB",
                                       bufs=1, name=f"t{jj}{half}")
                        for blk in range(4):
                            nc.tensor.transpose(
                                t_ps[:, 128 * blk:128 * blk + 128].bitcast(F32R),
                                vt_st[:, 128 * blk:128 * blk + 128], ident[:])
                        nc.vector.tensor_copy(
                            vnat[:, 4 * half:4 * half + 4, :]
                            .rearrange("p t (h x) -> p t h x", h=4)
                            [:, :, 2 * jj:2 * jj + 2, 0:64],
                            t_ps[:].rearrange("p (t h x) -> p t h x", t=4, h=2),
                        )

            # ---------------- P2: attention (+ per-head d broadcast) --------
            dstages = []
            for jj in range(2):
                dstages.append(sb.tile([128, 2048], F32, tag="ln1t", bufs=3,
                                       name=f"dst{jj}"))
            for hl in range(4):
                s_ps = ps.tile([128, 2048], F32, tag="tagA", bufs=1,
                               name=f"s{hl}")
                u_ps = ps.tile([65, 2048], F32, tag="tagB", bufs=1,
                               name=f"u{hl}")
                e_t = sb.tile([128, 2048], F32R, tag="e", bufs=1, name=f"e{hl}")
                for kc in range(16):
                    for u in range(4):
                        nc.tensor.matmul(
                            s_ps[:, 512 * u:512 * u + 512],
                            kt_sb[32 * hl:32 * hl + 32, 128 * kc:128 * kc + 128],
                            qt_sb[32 * hl:32 * hl + 32, 512 * u:512 * u + 512],
                            tile_position=(32 * hl, 0), start=True, stop=True)
                    nc.scalar.activation(e_t[:], s_ps[:], AF.Exp, scale=SCALE)
                    for u in range(4):
                        nc.tensor.matmul(
                            u_ps[:, 512 * u:512 * u + 512],
                            vnat[:, kc, 65 * hl:65 * hl + 65],
                            e_t[:, 512 * u:512 * u + 512],
                            start=(kc == 0), stop=(kc == 15))
                u_st = sb.tile([65, S], F32R, tag="ust", bufs=2, name=f"ust{hl}")
                nc.vector.tensor_copy(u_st[:], u_ps[:])
                nc.sync.dma_start(
                    ysb[64 * (hl % 2):64 * (hl % 2) + 64, hl // 2, :],
                    u_st[0:64, :])
                # broadcast this head's denominator row (at partition 64)
                db = ps.tile([128, 2048], F32, tag="tagA", bufs=1,
                             name=f"db{hl}")
                for u in range(4):
                    nc.tensor.matmul(
                        db[:, 512 * u:512 * u + 512], onesmat[64:65, :],
                        u_st[64:65, 512 * u:512 * u + 512], start=True, stop=True)
                half = hl % 2
                nc.vector.tensor_copy(
                    dstages[hl // 2][64 * half:64 * half + 64, :],
                    db[64 * half:64 * half + 64, :])

            # ---------------- P3: divide, LN1 stats + AR, normalize ----------
            for jj in range(2):
                rec_t = sb.tile([128, 2048], F32, tag="ln1t", bufs=3,
                                name=f"rec{jj}")
                nc.vector.reciprocal_approx_fast(rec_t[:], dstages[jj][:])
                nc.vector.tensor_tensor(ysb[:, jj, :], ut_sb[:, jj, :], rec_t[:],
                                        OP.mult)
            st_s = ps.tile([1, 2048], F32, tag="tagA", bufs=1, name="sts")
            st_q = ps.tile([1, 2048], F32, tag="tagB", bufs=1, name="stq")
            for jj in range(2):
                ysq = sb.tile([128, 2048], F32R, tag="ln1t", bufs=3,
                              name=f"ysq{jj}")
                nc.vector.tensor_tensor(ysq[:], ysb[:, jj, :], ysb[:, jj, :], OP.mult)
                for u in range(4):
                    usl = slice(512 * u, 512 * u + 512)
                    nc.tensor.matmul(st_s[0:1, usl], onescol[:], ysb[:, jj, usl],
                                     start=(jj == 0), stop=(jj == 1))
                    nc.tensor.matmul(st_q[0:1, usl], onescol[:], ysq[:, usl],
                                     start=(jj == 0), stop=(jj == 1))
            ss_st = sb.tile([1, 2048], F32R, tag="row", bufs=2, name="ssst")
            sq_st = sb.tile([1, 2048], F32R, tag="row", bufs=2, name="sqst")
            nc.vector.tensor_copy(ss_st[:], st_s[0:1, :])
            nc.vector.tensor_copy(sq_st[:], st_q[0:1, :])
            nc.sync.dma_start(stats_in[0:1, :], ss_st[:])
            nc.sync.dma_start(stats_in[1:2, :], sq_st[:])

        # collectives cannot live inside a For_i loop on this runtime
        # (NRT_EXEC_UNIT_UNRECOVERABLE), so the timing reps run them
        # unrolled between the loops; every rep's data is identical, so
        # results are unchanged and each rep still executes them once.
        for _ in range(rar):
            nc.gpsimd.collective_compute(
                "AllReduce", OP.add, replica_groups=GROUPS,
                ins=[stats_in[:]], outs=[stats_out[:]])

        with loop("reploop2", r2):
            str_s = sb.tile([1, 2048], F32R, tag="row", bufs=2, name="strs")
            str_q = sb.tile([1, 2048], F32R, tag="row", bufs=2, name="strq")
            nc.sync.dma_start(str_s[:], stats_out[0:1, :])
            nc.sync.dma_start(str_q[:], stats_out[1:2, :])

            bs_ps = ps.tile([128, 2048], F32, tag="tagA", bufs=1, name="bs")
            bq_ps = ps.tile([128, 2048], F32, tag="tagB", bufs=1, name="bq2")
            for u in range(4):
                usl = slice(512 * u, 512 * u + 512)
                nc.tensor.matmul(bs_ps[:, usl], onesrow[:], str_s[0:1, usl],
                                 start=True, stop=True)
                nc.tensor.matmul(bq_ps[:, usl], onesrow[:], str_q[0:1, usl],
                                 start=True, stop=True)
            t_mu = sb.tile([128, 2048], F32, tag="ln1t", bufs=3, name="tmu")
            t_v = sb.tile([128, 2048], F32, tag="ln1t", bufs=3, name="tv")
            t_w = sb.tile([128, 2048], F32, tag="ln1t", bufs=3, name="tw")
            nc.vector.tensor_scalar(t_mu[:], bs_ps[:], 1.0 / D, None, OP.mult)
            nc.vector.tensor_scalar(t_v[:], bq_ps[:], 1.0 / D, None, OP.mult)
            nc.vector.tensor_tensor(t_w[:], t_mu[:], t_mu[:], OP.mult)
            nc.vector.tensor_tensor(t_v[:], t_v[:], t_w[:], OP.subtract)
            nc.vector.tensor_scalar(t_v[:], t_v[:], EPS, None, OP.add)
            nc.vector.reciprocal_approx_fast(t_w[:], t_v[:])
            nc.scalar.activation(t_v[:], t_w[:], AF.Sqrt)                 # r
            nc.vector.tensor_tensor(t_w[:], t_mu[:], t_v[:], OP.mult)     # mu*r
            for jj in range(2):
                nc.vector.tensor_tensor(yn[:, jj, :], ut_sb[:, jj, :], t_v[:], OP.mult)
                nc.vector.tensor_tensor(yn[:, jj, :], ynf[:, jj, :], t_w[:],
                                        OP.subtract)
                nc.vector.tensor_scalar(yn[:, jj, :], ynf[:, jj, :],
                                        g1[:, jj:jj + 1], b1[:, jj:jj + 1],
                                        OP.mult, OP.add)

            # ---------------- fc + RS ----------------
            for pair in range(8):       # 2 token-chunks of 128 per psum tile
                fc_ps = ps.tile([128, 2048], F32,
                                tag="tagA" if pair % 2 == 0 else "tagB",
                                bufs=1, name=f"fc{pair}")
                for half in range(2):
                    tok = slice(256 * pair + 128 * half, 256 * pair + 128 * half + 128)
                    for jj in range(2):
                        for nch in range(2):
                            nc.tensor.matmul(
                                fc_ps[:, 1024 * half + 512 * nch:
                                      1024 * half + 512 * nch + 512],
                                yn[:, jj, tok],
                                wfc[:, jj, 512 * nch:512 * nch + 512],
                                start=(jj == 0), stop=(jj == 1))
                p_st = sb.tile([128, 2048], F32, tag="pst", bufs=1,
                               name=f"pst{pair}")
                nc.vector.tensor_copy(p_st[:], fc_ps[:])
                nc.sync.dma_start(
                    rs_in[256 * pair:256 * pair + 256, :]
                    .rearrange("(t p) n -> p t n", t=2),
                    p_st[:].rearrange("p (t n) -> p t n", t=2))

        for _ in range(rrs):
            nc.gpsimd.collective_compute(
                "ReduceScatter", OP.add, replica_groups=GROUPS,
                ins=[rs_in[:]], outs=[rs_out[:]])

        # ---------------- LN2 ----------------
        with loop("reploop3", r3):
            for ts in range(4):
                pP = sb.tile([128, D], F32, tag="pP", bufs=1, name=f"pP{ts}")
                nc.sync.dma_start(pP[:], rs_out[128 * ts:128 * ts + 128, :])
                nc.vector.tensor_tensor(pP[:], pP[:], fcbc[:], OP.add)
                s2 = sb.tile([128, 8], F32, tag="s2", bufs=2, name=f"s2{ts}")
                nc.vector.tensor_reduce(s2[:, 0:1], pP[:], AX.X, OP.add)
                sqd = sb.tile([128, D], F32, tag="ln1t", bufs=3, name=f"sqd{ts}")
                nc.scalar.activation(sqd[:], pP[:], AF.Square, accum_out=s2[:, 1:2])
                nc.vector.tensor_scalar(s2[:, 0:1], s2[:, 0:1], 1.0 / D, None, OP.mult)
                nc.vector.tensor_scalar(s2[:, 1:2], s2[:, 1:2], 1.0 / D, None, OP.mult)
                nc.vector.tensor_tensor(s2[:, 2:3], s2[:, 0:1], s2[:, 0:1], OP.mult)
                nc.vector.tensor_tensor(s2[:, 3:4], s2[:, 1:2], s2[:, 2:3], OP.subtract)
                nc.vector.tensor_scalar(s2[:, 3:4], s2[:, 3:4], EPS, None, OP.add)
                nc.vector.reciprocal_approx_fast(s2[:, 4:5], s2[:, 3:4])
                nc.scalar.activation(s2[:, 5:6], s2[:, 4:5], AF.Sqrt)
                nc.vector.tensor_scalar(pP[:], pP[:], s2[:, 0:1], s2[:, 5:6],
                                        OP.subtract, OP.mult)
                nc.vector.tensor_tensor(pP[:], pP[:], g2bc[:], OP.mult)
                nc.vector.tensor_tensor(pP[:], pP[:], b2bc[:], OP.add)
                nc.sync.dma_start(out_d[ts], pP[:])

    nc.compile()
    _BUILD_CACHE[key] = nc
    return nc


def make_in_maps(x, Wq, bq, Wk, bk, Wv, bv, gamma1, beta1, Wfc, bfc, gamma2, beta2):
    x = np.asarray(x, np.float32)
    in_maps = []
    onesrow = np.ones((1, 128), np.float32)
    onesmat = np.ones((128, 128), np.float32)
    onescol = np.ones((128, 1), np.float32)
    onesv = np.ones((128, 64), np.float32)
    ident = np.eye(128, dtype=np.float32)
    Wq, Wk, Wv, Wfc = (np.asarray(a, np.float32) for a in (Wq, Wk, Wv, Wfc))
    for c in range(8):
        g, r = c // 4, c % 4
        xt = np.ascontiguousarray(
            x[g].T.reshape(8, 128, S).transpose(1, 0, 2))          # [128, 8, S]
        wq_c = np.ascontiguousarray(
            Wq[:, 128 * r:128 * r + 128].reshape(8, 128, 128).transpose(1, 0, 2))
        wk_c = np.ascontiguousarray(
            Wk[:, 128 * r:128 * r + 128].reshape(8, 128, 128).transpose(1, 0, 2))
        wv_c = np.ascontiguousarray(
            Wv[:, 256 * r:256 * r + 256].reshape(8, 128, 256).transpose(1, 0, 2))
        wfc_c = np.ascontiguousarray(
            Wfc[256 * r:256 * r + 256, :].reshape(2, 128, D).transpose(1, 0, 2))
        in_maps.append({
            "xt": xt, "wq": wq_c, "wk": wk_c, "wv": wv_c, "wfc": wfc_c,
            "onesrow": onesrow, "onesmat": onesmat, "onescol": onescol,
            "onesv": onesv, "ident": ident,
            "bq": np.asarray(bq, np.float32)[128 * r:128 * r + 128, None],
            "bk": np.asarray(bk, np.float32)[128 * r:128 * r + 128, None],
            "bv": np.asarray(bv, np.float32)[256 * r:256 * r + 256]
                 .reshape(2, 128).T.copy(),
            "g1": np.asarray(gamma1, np.float32)[256 * r:256 * r + 256]
                 .reshape(2, 128).T.copy(),
            "b1": np.asarray(beta1, np.float32)[256 * r:256 * r + 256]
                 .reshape(2, 128).T.copy(),
            "g2row": np.asarray(gamma2, np.float32)[None, :].copy(),
            "b2row": np.asarray(beta2, np.float32)[None, :].copy(),
            "bfcrow": np.asarray(bfc, np.float32)[None, :].copy(),
        })
    return in_maps


def assemble(results):
    out = np.empty((B, S, D), np.float32)
    for c in range(8):
        g, r = c // 4, c % 4
        o = results[c]["out"]                   # [4, 128, D] = slab r of batch g
        for ts in range(4):
            out[g, 512 * r + 128 * ts:512 * r + 128 * ts + 128, :] = o[ts]
    return out


def kernel(**inputs):
    nc = _build()
    in_maps = make_in_maps(**{k: np.asarray(v) for k, v in inputs.items()})
    res = run_bass_kernel_spmd(nc, in_maps, list(range(8)))
    return assemble(res.results)
